# revision 11
# baseline (speedup 1.0000x reference)
"""Multi-head attention (B=8, N=1024, C=768, H=12, D=64) on 8 TRN2 NeuronCores.

Sharding: pure data parallel — one batch element per core, weights replicated,
no collectives. Each core computes its full attention block.

On-chip layout strategy (per core):
  - x is PE-transposed to x^T [C, N] (feature-major) once.
  - qk^T [1536, N] = w_qkv[:, :1536].T @ x^T   (transposed activations)
  - v [N, 768] natural = (x^T).T @ w_qkv[:, 1536:], stored with a ones column
    appended per head ([128, 12, 65] tiles) so attn@v also yields the softmax
    denominator (row 64 of the output) in the same matmul.
  - per head: scores^T [m, n] = k^T.T @ q^T (K=64), exp on ACT with the
    1/sqrt(D) scale folded in (softmax without max-subtraction: logits are
    O(10), far below fp32 exp overflow), out_aug^T [65, n] = v_aug.T @ exp,
    reciprocal of row 64, PE-broadcast of the reciprocal across partitions,
    DVE multiply -> normalized out^T [C, N].
  - final = (out^T).T @ w_proj + b: out^T is the stationary operand, which
    flips the result back to natural [N, C] layout for a contiguous DMA out.
All matmuls run as float32r (full fp32 precision, 1 cycle/row at N>=256).
"""

import numpy as np

B, N, C = 8, 1024, 768
H, D = 12, 64
F3 = 3 * C          # 2304
FQK = 2 * C         # 1536
SCALE = D ** -0.5   # 0.125
NT = N // 128       # 8 n-tiles
CT = C // 128       # 6 c-tiles
FT = FQK // 128     # 12 qk feature tiles
NCH = N // 512      # 2 psum chunks over n
VCH = 384           # v / proj free chunk (C = 2*384)

_compiled = None


def _build():
    import concourse.bass as bass
    import concourse.mybir as mybir
    import concourse.tile as tile
    from concourse import bacc
    from concourse.masks import make_identity

    f32 = mybir.dt.float32
    f32r = mybir.dt.float32r

    nc = bacc.Bacc("TRN2", target_bir_lowering=False, debug=False)

    x_d = nc.dram_tensor("x", [N, C], f32, kind="ExternalInput").ap()
    wqkv_d = nc.dram_tensor("w_qkv", [C, F3], f32r, kind="ExternalInput").ap()
    wproj_d = nc.dram_tensor("w_proj", [C, C], f32r, kind="ExternalInput").ap()
    bias_d = nc.dram_tensor("b_bcast", [128, C], f32, kind="ExternalInput").ap()
    out_d = nc.dram_tensor("out", [N, C], f32, kind="ExternalOutput").ap()

    def r(ap):
        return ap

    with tile.TileContext(nc) as tc:
        with tc.tile_pool(name="const", bufs=1) as const_pool:
            ident = const_pool.tile([128, 128], f32)
            make_identity(nc, ident[:])
            ones_f32 = const_pool.tile([1, 128], f32)
            nc.gpsimd.memset(ones_f32[:], 1.0)
            ones_sb = const_pool.tile([1, 128], f32r)
            nc.vector.tensor_copy(ones_sb[:], ones_f32[:])
            vones_f32 = const_pool.tile([128, H], f32)
            nc.gpsimd.memset(vones_f32[:], 1.0)
            bias_sb = const_pool.tile([128, C], f32)
            nc.sync.dma_start(bias_sb[:], bias_d)

            # ---- persistent activations ----
            with tc.tile_pool(name="acts", bufs=1) as acts:
                xT = [acts.tile([128, N], f32r, tag=f"xT{ci}", name=f"xT{ci}") for ci in range(CT)]
                qkT = [acts.tile([128, N], f32r, tag=f"qkT{fi}", name=f"qkT{fi}") for fi in range(FT)]
                vnat = [acts.tile([128, H, D + 1], f32r, tag=f"v{ni}", name=f"v{ni}")
                        for ni in range(NT)]
                onorm = [acts.tile([128, N], f32r, tag=f"on{ci}", name=f"on{ci}") for ci in range(CT)]

                # ---- phase 0: load x, transpose to x^T ----
                with tc.tile_pool(name="xin", bufs=3) as xin_pool, \
                     tc.tile_pool(name="pst", bufs=2, space="PSUM") as pst_pool, \
                     tc.tile_pool(name="wq", bufs=1) as wq_pool, \
                     tc.tile_pool(name="psqk", bufs=2, space="PSUM") as psqk_pool, \
                     tc.tile_pool(name="psv", bufs=2, space="PSUM") as psv_pool:
                    wq = [wq_pool.tile([128, F3], f32r, tag=f"wq{ci}", name=f"wq{ci}")
                          for ci in range(CT)]
                    for ci in range(CT):
                        nc.sync.dma_start(
                            wq[ci][:], wqkv_d[ci * 128:(ci + 1) * 128, :])

                    for ni in range(NT):
                        xt_in = xin_pool.tile([128, C], f32)
                        nc.sync.dma_start(
                            xt_in[:], x_d[ni * 128:(ni + 1) * 128, :])
                        for ci in range(CT):
                            pt = pst_pool.tile([128, 128], f32)
                            nc.tensor.transpose(
                                pt[:], xt_in[:, ci * 128:(ci + 1) * 128],
                                ident[:])
                            nc.vector.tensor_copy(
                                xT[ci][:, ni * 128:(ni + 1) * 128], pt[:])

                    # ---- phase A1: qk^T = w_qk.T @ x^T ----
                    for fi in range(FT):
                        for ch in range(NCH):
                            pqk = psqk_pool.tile([128, 512], f32)
                            for ci in range(CT):
                                nc.tensor.matmul(
                                    pqk[:],
                                    r(wq[ci][:, fi * 128:(fi + 1) * 128]),
                                    r(xT[ci][:, ch * 512:(ch + 1) * 512]),
                                    start=(ci == 0), stop=(ci == CT - 1))
                            nc.vector.tensor_copy(
                                qkT[fi][:, ch * 512:(ch + 1) * 512], pqk[:])

                    # ---- phase A2: v natural = (x^T).T @ w_v, + ones col ----
                    for ni in range(NT):
                        nc.vector.tensor_copy(vnat[ni][:, :, D], vones_f32[:])
                        for vc in range(2):
                            pv = psv_pool.tile([128, VCH], f32)
                            for ci in range(CT):
                                nc.tensor.matmul(
                                    pv[:],
                                    r(xT[ci][:, ni * 128:(ni + 1) * 128]),
                                    r(wq[ci][:, FQK + vc * VCH:
                                             FQK + (vc + 1) * VCH]),
                                    start=(ci == 0), stop=(ci == CT - 1))
                            nc.vector.tensor_copy(
                                vnat[ni][:, vc * 6:(vc + 1) * 6, 0:D],
                                pv[:].rearrange("p (h d) -> p h d", d=D))

                # ---- phase B: attention per head ----
                with tc.tile_pool(name="wp", bufs=1) as wp_pool:
                    wp = [wp_pool.tile([128, C], f32r, tag=f"wp{ci}", name=f"wp{ci}")
                          for ci in range(CT)]
                    for ci in range(CT):
                        nc.sync.dma_start(
                            wp[ci][:], wproj_d[ci * 128:(ci + 1) * 128, :])

                  # (indent fix below)
                    attn_pools = (
                        tc.tile_pool(name="rc", bufs=4),
                        tc.tile_pool(name="exp", bufs=10),
                        tc.tile_pool(name="pss", bufs=3, space="PSUM"),
                        tc.tile_pool(name="pso", bufs=2, space="PSUM"),
                        tc.tile_pool(name="psb", bufs=2, space="PSUM"),
                    )
                    rc_pool, exp_pool, pss_pool, pso_pool, psb_pool = [
                        p.__enter__() for p in attn_pools]

                    for h in range(H):
                        qrow = (h % 2) * D
                        qT_h = qkT[h // 2][qrow:qrow + D, :]
                        kT_h = qkT[6 + h // 2][qrow:qrow + D, :]

                        exp_t = []
                        for mi in range(NT):
                            et = exp_pool.tile([128, N], f32r, tag="exp", name=f"exp{mi}")
                            exp_t.append(et)
                            for ch in range(NCH):
                                ps = pss_pool.tile([128, 512], f32)
                                nc.tensor.matmul(
                                    ps[:],
                                    r(kT_h[:, mi * 128:(mi + 1) * 128]),
                                    r(qT_h[:, ch * 512:(ch + 1) * 512]),
                                    start=True, stop=True)
                                nc.scalar.activation(
                                    et[:, ch * 512:(ch + 1) * 512], ps[:],
                                    mybir.ActivationFunctionType.Exp,
                                    scale=SCALE)

                        orow = (h % 2) * D
                        for ch in range(NCH):
                            po = pso_pool.tile([D + 1, 512], f32)
                            for mi in range(NT):
                                nc.tensor.matmul(
                                    po[:],
                                    r(vnat[mi][:, h, :]),
                                    r(exp_t[mi][:, ch * 512:(ch + 1) * 512]),
                                    start=(mi == 0), stop=(mi == NT - 1))
                            rc = rc_pool.tile([1, 512], f32r, tag="rc",
                                              name=f"rc{h}_{ch}")
                            with nc.allow_low_precision(
                                    reason="f32r denominators, full fp32 value range"):
                                nc.vector.reciprocal(rc[:], po[D:D + 1, :])
                            pb = psb_pool.tile([128, 512], f32)
                            nc.tensor.matmul(
                                pb[:], r(ones_sb[:]), r(rc[:]),
                                start=True, stop=True)
                            ou = rc_pool.tile([D, 512], f32, tag="ou",
                                              name=f"ou{h}_{ch}")
                            nc.vector.tensor_copy(ou[:], po[0:D, :])
                            nc.vector.tensor_mul(
                                onorm[h // 2][orow:orow + D,
                                              ch * 512:(ch + 1) * 512],
                                ou[:], pb[0:D, :])

                    for p in reversed(attn_pools):
                        p.__exit__(None, None, None)

                    # ---- phase C: final = (out^T).T @ w_proj + b ----
                    with tc.tile_pool(name="fin", bufs=3) as fin_pool, \
                         tc.tile_pool(name="psf", bufs=2, space="PSUM") as psf_pool:
                        for ni in range(NT):
                            fin = fin_pool.tile([128, C], f32)
                            for fc in range(2):
                                pf = psf_pool.tile([128, VCH], f32)
                                for ci in range(CT):
                                    nc.tensor.matmul(
                                        pf[:],
                                        r(onorm[ci][:, ni * 128:(ni + 1) * 128]),
                                        r(wp[ci][:, fc * VCH:(fc + 1) * VCH]),
                                        start=(ci == 0), stop=(ci == CT - 1))
                                nc.vector.tensor_add(
                                    fin[:, fc * VCH:(fc + 1) * VCH], pf[:],
                                    bias_sb[:, fc * VCH:(fc + 1) * VCH])
                            nc.sync.dma_start(
                                out_d[ni * 128:(ni + 1) * 128, :], fin[:])

    nc.compile()
    return nc


def _get_compiled():
    global _compiled
    if _compiled is None:
        _compiled = _build()
    return _compiled


def _run(x, w_qkv, w_proj, b_proj, **kwargs):
    from concourse.bass_utils import run_bass_kernel_spmd

    x = np.asarray(x, dtype=np.float32)
    w_qkv = np.ascontiguousarray(np.asarray(w_qkv, dtype=np.float32))
    w_proj = np.ascontiguousarray(np.asarray(w_proj, dtype=np.float32))
    b_bcast = np.ascontiguousarray(
        np.broadcast_to(np.asarray(b_proj, dtype=np.float32), (128, C)))

    nc = _get_compiled()
    in_maps = [
        {"x": np.ascontiguousarray(x[b]), "w_qkv": w_qkv,
         "w_proj": w_proj, "b_bcast": b_bcast}
        for b in range(B)
    ]
    return run_bass_kernel_spmd(nc, in_maps, core_ids=list(range(B)), **kwargs)


def kernel(x, w_qkv, w_proj, b_proj, **_):
    res = _run(x, w_qkv, w_proj, b_proj)
    return np.stack([res.results[b]["out"] for b in range(B)], axis=0)


# revision 14
# speedup vs baseline: 1.0818x; 1.0818x over previous
"""Multi-head attention (B=8, N=1024, C=768, H=12, D=64) on 8 TRN2 NeuronCores.

Sharding: pure data parallel — one batch element per core, weights replicated,
no collectives. Each core computes its full attention block.

On-chip layout strategy (per core):
  - x is PE-transposed to x^T [C, N] (feature-major) once.
  - qk^T [1536, N] = w_qkv[:, :1536].T @ x^T   (transposed activations)
  - v [N, 768] natural = (x^T).T @ w_qkv[:, 1536:], stored with a ones column
    appended per head ([128, 12, 65] tiles) so attn@v also yields the softmax
    denominator (row 64 of the output) in the same matmul.
  - per head: scores^T [m, n] = k^T.T @ q^T (K=64), exp on ACT with the
    1/sqrt(D) scale folded in (softmax without max-subtraction: logits are
    O(10), far below fp32 exp overflow), out_aug^T [65, n] = v_aug.T @ exp,
    reciprocal of row 64, PE-broadcast of the reciprocal across partitions,
    DVE multiply -> normalized out^T [C, N].
  - final = (out^T).T @ w_proj + b: out^T is the stationary operand, which
    flips the result back to natural [N, C] layout for a contiguous DMA out.
All matmuls run as float32r (full fp32 precision, 1 cycle/row at N>=256).
"""

import numpy as np

B, N, C = 8, 1024, 768
H, D = 12, 64
F3 = 3 * C          # 2304
FQK = 2 * C         # 1536
SCALE = D ** -0.5   # 0.125
NT = N // 128       # 8 n-tiles
CT = C // 128       # 6 c-tiles
FT = FQK // 128     # 12 qk feature tiles
NCH = N // 512      # 2 psum chunks over n
VCH = 384           # v / proj free chunk (C = 2*384)

_compiled = None


def _build():
    import concourse.bass as bass
    import concourse.mybir as mybir
    import concourse.tile as tile
    from concourse import bacc
    from concourse.masks import make_identity

    f32 = mybir.dt.float32
    f32r = mybir.dt.float32r

    nc = bacc.Bacc("TRN2", target_bir_lowering=False, debug=False)

    x_d = nc.dram_tensor("x", [N, C], f32, kind="ExternalInput").ap()
    wqkv_d = nc.dram_tensor("w_qkv", [C, F3], f32r, kind="ExternalInput").ap()
    wproj_d = nc.dram_tensor("w_proj", [C, C], f32r, kind="ExternalInput").ap()
    bias_d = nc.dram_tensor("b_bcast", [128, C], f32, kind="ExternalInput").ap()
    out_d = nc.dram_tensor("out", [N, C], f32, kind="ExternalOutput").ap()

    def r(ap):
        return ap

    with tile.TileContext(nc) as tc:
        with tc.tile_pool(name="const", bufs=1) as const_pool:
            ident = const_pool.tile([128, 128], f32)
            make_identity(nc, ident[:])
            ones_f32 = const_pool.tile([1, 128], f32)
            nc.gpsimd.memset(ones_f32[:], 1.0)
            ones_sb = const_pool.tile([1, 128], f32r)
            nc.vector.tensor_copy(ones_sb[:], ones_f32[:])
            vones_f32 = const_pool.tile([128, H], f32)
            nc.gpsimd.memset(vones_f32[:], 1.0)
            bias_sb = const_pool.tile([128, C], f32)
            nc.scalar.dma_start(bias_sb[:], bias_d)

            # ---- persistent activations ----
            with tc.tile_pool(name="acts", bufs=1) as acts:
                xT = [acts.tile([128, N], f32r, tag=f"xT{ci}", name=f"xT{ci}") for ci in range(CT)]
                qkT = [acts.tile([128, N], f32r, tag=f"qkT{fi}", name=f"qkT{fi}") for fi in range(FT)]
                vnat = [acts.tile([128, H, D + 1], f32r, tag=f"v{ni}", name=f"v{ni}")
                        for ni in range(NT)]
                onorm = [acts.tile([128, N], f32r, tag=f"on{ci}", name=f"on{ci}") for ci in range(CT)]

                # ---- phase 0: load x, transpose to x^T ----
                with tc.tile_pool(name="xin", bufs=4) as xin_pool, \
                     tc.tile_pool(name="pst", bufs=2, space="PSUM") as pst_pool, \
                     tc.tile_pool(name="wq", bufs=1) as wq_pool, \
                     tc.tile_pool(name="psqk", bufs=2, space="PSUM") as psqk_pool, \
                     tc.tile_pool(name="psv", bufs=2, space="PSUM") as psv_pool:
                    wq = [wq_pool.tile([128, F3], f32r, tag=f"wq{ci}", name=f"wq{ci}")
                          for ci in range(CT)]
                    xt_ins = []
                    for ni in range(NT):
                        xt_in = xin_pool.tile([128, C], f32, tag="xt_in",
                                              name=f"xt_in{ni}")
                        xt_ins.append(xt_in)
                        nc.sync.dma_start(
                            xt_in[:], x_d[ni * 128:(ni + 1) * 128, :])
                    for ci in range(CT):
                        nc.scalar.dma_start(
                            wq[ci][:], wqkv_d[ci * 128:(ci + 1) * 128, :])

                    for ni in range(NT):
                        xt_in = xt_ins[ni]
                        for ci in range(CT):
                            pt = pst_pool.tile([128, 128], f32)
                            nc.tensor.transpose(
                                pt[:], xt_in[:, ci * 128:(ci + 1) * 128],
                                ident[:])
                            nc.vector.tensor_copy(
                                xT[ci][:, ni * 128:(ni + 1) * 128], pt[:])

                    # ---- phase A1: qk^T = w_qk.T @ x^T ----
                    for fi in range(FT):
                        for ch in range(NCH):
                            pqk = psqk_pool.tile([128, 512], f32)
                            for ci in range(CT):
                                nc.tensor.matmul(
                                    pqk[:],
                                    r(wq[ci][:, fi * 128:(fi + 1) * 128]),
                                    r(xT[ci][:, ch * 512:(ch + 1) * 512]),
                                    start=(ci == 0), stop=(ci == CT - 1))
                            nc.vector.tensor_copy(
                                qkT[fi][:, ch * 512:(ch + 1) * 512], pqk[:])

                    # ---- phase A2: v natural = (x^T).T @ w_v, + ones col ----
                    for ni in range(NT):
                        nc.vector.tensor_copy(vnat[ni][:, :, D], vones_f32[:])
                        for vc in range(2):
                            pv = psv_pool.tile([128, VCH], f32)
                            for ci in range(CT):
                                nc.tensor.matmul(
                                    pv[:],
                                    r(xT[ci][:, ni * 128:(ni + 1) * 128]),
                                    r(wq[ci][:, FQK + vc * VCH:
                                             FQK + (vc + 1) * VCH]),
                                    start=(ci == 0), stop=(ci == CT - 1))
                            nc.vector.tensor_copy(
                                vnat[ni][:, vc * 6:(vc + 1) * 6, 0:D],
                                pv[:].rearrange("p (h d) -> p h d", d=D))

                # ---- phase B: attention per head ----
                with tc.tile_pool(name="wp", bufs=1) as wp_pool:
                    wp = [wp_pool.tile([128, C], f32r, tag=f"wp{ci}", name=f"wp{ci}")
                          for ci in range(CT)]
                    for ci in range(CT):
                        nc.scalar.dma_start(
                            wp[ci][:], wproj_d[ci * 128:(ci + 1) * 128, :])

                  # (indent fix below)
                    attn_pools = (
                        tc.tile_pool(name="rc", bufs=4),
                        tc.tile_pool(name="exp", bufs=10),
                        tc.tile_pool(name="pss", bufs=3, space="PSUM"),
                        tc.tile_pool(name="pso", bufs=3, space="PSUM"),
                        tc.tile_pool(name="psb", bufs=2, space="PSUM"),
                    )
                    rc_pool, exp_pool, pss_pool, pso_pool, psb_pool = [
                        p.__enter__() for p in attn_pools]

                    for h in range(H):
                        qrow = (h % 2) * D
                        qT_h = qkT[h // 2][qrow:qrow + D, :]
                        kT_h = qkT[6 + h // 2][qrow:qrow + D, :]

                        exp_t = []
                        for mi in range(NT):
                            et = exp_pool.tile([128, N], f32r, tag="exp", name=f"exp{mi}")
                            exp_t.append(et)
                            for ch in range(NCH):
                                ps = pss_pool.tile([128, 512], f32)
                                nc.tensor.matmul(
                                    ps[:],
                                    r(kT_h[:, mi * 128:(mi + 1) * 128]),
                                    r(qT_h[:, ch * 512:(ch + 1) * 512]),
                                    start=True, stop=True)
                                nc.scalar.activation(
                                    et[:, ch * 512:(ch + 1) * 512], ps[:],
                                    mybir.ActivationFunctionType.Exp,
                                    scale=SCALE)

                        orow = (h % 2) * D
                        for ch in range(NCH):
                            po = pso_pool.tile([D + 1, 512], f32)
                            for mi in range(NT):
                                nc.tensor.matmul(
                                    po[:],
                                    r(vnat[mi][:, h, :]),
                                    r(exp_t[mi][:, ch * 512:(ch + 1) * 512]),
                                    start=(mi == 0), stop=(mi == NT - 1))
                            rs = rc_pool.tile([1, 512], f32, tag="rs",
                                              name=f"rs{h}_{ch}", bufs=2)
                            nc.vector.tensor_copy(rs[:], po[D:D + 1, :])
                            rcf = rc_pool.tile([1, 512], f32, tag="rcf",
                                               name=f"rcf{h}_{ch}", bufs=2)
                            nc.vector.reciprocal_approx_fast(rcf[:], rs[:])
                            rc = rc_pool.tile([1, 512], f32r, tag="rc",
                                              name=f"rc{h}_{ch}", bufs=2)
                            nc.vector.tensor_copy(rc[:], rcf[:])
                            pb = psb_pool.tile([128, 512], f32)
                            nc.tensor.matmul(
                                pb[:], r(ones_sb[:]), r(rc[:]),
                                start=True, stop=True)
                            ou = rc_pool.tile([D, 512], f32, tag="ou",
                                              name=f"ou{h}_{ch}", bufs=3)
                            nc.vector.tensor_copy(ou[:], po[0:D, :])
                            nc.vector.tensor_mul(
                                onorm[h // 2][orow:orow + D,
                                              ch * 512:(ch + 1) * 512],
                                ou[:], pb[0:D, :])

                    for p in reversed(attn_pools):
                        p.__exit__(None, None, None)

                    # ---- phase C: final = (out^T).T @ w_proj + b ----
                    with tc.tile_pool(name="fin", bufs=3) as fin_pool, \
                         tc.tile_pool(name="psf", bufs=2, space="PSUM") as psf_pool:
                        for ni in range(NT):
                            fin = fin_pool.tile([128, C], f32)
                            for fc in range(2):
                                pf = psf_pool.tile([128, VCH], f32)
                                for ci in range(CT):
                                    nc.tensor.matmul(
                                        pf[:],
                                        r(onorm[ci][:, ni * 128:(ni + 1) * 128]),
                                        r(wp[ci][:, fc * VCH:(fc + 1) * VCH]),
                                        start=(ci == 0), stop=(ci == CT - 1))
                                nc.vector.tensor_add(
                                    fin[:, fc * VCH:(fc + 1) * VCH], pf[:],
                                    bias_sb[:, fc * VCH:(fc + 1) * VCH])
                            nc.sync.dma_start(
                                out_d[ni * 128:(ni + 1) * 128, :], fin[:])

    nc.compile()
    return nc


def _get_compiled():
    global _compiled
    if _compiled is None:
        _compiled = _build()
    return _compiled


def _run(x, w_qkv, w_proj, b_proj, **kwargs):
    from concourse.bass_utils import run_bass_kernel_spmd

    x = np.asarray(x, dtype=np.float32)
    w_qkv = np.ascontiguousarray(np.asarray(w_qkv, dtype=np.float32))
    w_proj = np.ascontiguousarray(np.asarray(w_proj, dtype=np.float32))
    b_bcast = np.ascontiguousarray(
        np.broadcast_to(np.asarray(b_proj, dtype=np.float32), (128, C)))

    nc = _get_compiled()
    in_maps = [
        {"x": np.ascontiguousarray(x[b]), "w_qkv": w_qkv,
         "w_proj": w_proj, "b_bcast": b_bcast}
        for b in range(B)
    ]
    return run_bass_kernel_spmd(nc, in_maps, core_ids=list(range(B)), **kwargs)


def kernel(x, w_qkv, w_proj, b_proj, **_):
    res = _run(x, w_qkv, w_proj, b_proj)
    return np.stack([res.results[b]["out"] for b in range(B)], axis=0)


# revision 15
# speedup vs baseline: 1.1929x; 1.1027x over previous
"""Multi-head attention (B=8, N=1024, C=768, H=12, D=64) on 8 TRN2 NeuronCores.

Sharding: pure data parallel — one batch element per core, weights replicated,
no collectives. Each core computes its full attention block.

On-chip layout strategy (per core):
  - x is PE-transposed to x^T [C, N] (feature-major) once.
  - qk^T [1536, N] = w_qkv[:, :1536].T @ x^T   (transposed activations)
  - v [N, 768] natural = (x^T).T @ w_qkv[:, 1536:], stored with a ones column
    appended per head ([128, 12, 65] tiles) so attn@v also yields the softmax
    denominator (row 64 of the output) in the same matmul.
  - per head: scores^T [m, n] = k^T.T @ q^T (K=64), exp on ACT with the
    1/sqrt(D) scale folded in (softmax without max-subtraction: logits are
    O(10), far below fp32 exp overflow), out_aug^T [65, n] = v_aug.T @ exp,
    reciprocal of row 64, PE-broadcast of the reciprocal across partitions,
    DVE multiply -> normalized out^T [C, N].
  - final = (out^T).T @ w_proj + b: out^T is the stationary operand, which
    flips the result back to natural [N, C] layout for a contiguous DMA out.
All matmuls run as float32r (full fp32 precision, 1 cycle/row at N>=256).
"""

import numpy as np

B, N, C = 8, 1024, 768
H, D = 12, 64
F3 = 3 * C          # 2304
FQK = 2 * C         # 1536
SCALE = D ** -0.5   # 0.125
NT = N // 128       # 8 n-tiles
CT = C // 128       # 6 c-tiles
FT = FQK // 128     # 12 qk feature tiles
NCH = N // 512      # 2 psum chunks over n
VCH = 384           # v / proj free chunk (C = 2*384)

_compiled = None


def _build():
    import concourse.bass as bass
    import concourse.mybir as mybir
    import concourse.tile as tile
    from concourse import bacc
    from concourse.masks import make_identity

    f32 = mybir.dt.float32
    f32r = mybir.dt.float32r
    f16 = mybir.dt.float16

    nc = bacc.Bacc("TRN2", target_bir_lowering=False, debug=False)

    x_d = nc.dram_tensor("x", [N, C], f32, kind="ExternalInput").ap()
    wqkv_d = nc.dram_tensor("w_qkv", [C, F3], f32r, kind="ExternalInput").ap()
    wproj_d = nc.dram_tensor("w_proj", [C, C], f32r, kind="ExternalInput").ap()
    bias_d = nc.dram_tensor("b_bcast", [128, C], f32, kind="ExternalInput").ap()
    out_d = nc.dram_tensor("out", [N, C], f32, kind="ExternalOutput").ap()

    def r(ap):
        return ap

    with tile.TileContext(nc) as tc:
        with tc.tile_pool(name="const", bufs=1) as const_pool:
            ident = const_pool.tile([128, 128], f32)
            make_identity(nc, ident[:])
            ones_f32 = const_pool.tile([1, 128], f32)
            nc.gpsimd.memset(ones_f32[:], 1.0)
            ones_sb = const_pool.tile([1, 128], f32r)
            nc.vector.tensor_copy(ones_sb[:], ones_f32[:])
            vones_f32 = const_pool.tile([128, H], f32)
            nc.gpsimd.memset(vones_f32[:], 1.0)
            bias_sb = const_pool.tile([128, C], f32)
            nc.scalar.dma_start(bias_sb[:], bias_d)

            # ---- persistent activations ----
            with tc.tile_pool(name="acts", bufs=1) as acts:
                xT = [acts.tile([128, N], f32r, tag=f"xT{ci}", name=f"xT{ci}") for ci in range(CT)]
                qkT = [acts.tile([128, N], f16, tag=f"qkT{fi}", name=f"qkT{fi}") for fi in range(FT)]
                vnat = [acts.tile([128, H, D + 1], f16, tag=f"v{ni}", name=f"v{ni}")
                        for ni in range(NT)]
                onorm = [acts.tile([128, N], f32r, tag=f"on{ci}", name=f"on{ci}") for ci in range(CT)]

                # ---- phase 0: load x, transpose to x^T ----
                with tc.tile_pool(name="xin", bufs=4) as xin_pool, \
                     tc.tile_pool(name="pst", bufs=2, space="PSUM") as pst_pool, \
                     tc.tile_pool(name="wq", bufs=1) as wq_pool, \
                     tc.tile_pool(name="psqk", bufs=2, space="PSUM") as psqk_pool, \
                     tc.tile_pool(name="psv", bufs=2, space="PSUM") as psv_pool:
                    wq = [wq_pool.tile([128, F3], f32r, tag=f"wq{ci}", name=f"wq{ci}")
                          for ci in range(CT)]
                    xt_ins = []
                    for ni in range(NT):
                        xt_in = xin_pool.tile([128, C], f32, tag="xt_in",
                                              name=f"xt_in{ni}")
                        xt_ins.append(xt_in)
                        nc.sync.dma_start(
                            xt_in[:], x_d[ni * 128:(ni + 1) * 128, :])
                    for ci in range(CT):
                        nc.scalar.dma_start(
                            wq[ci][:], wqkv_d[ci * 128:(ci + 1) * 128, :])

                    for ni in range(NT):
                        xt_in = xt_ins[ni]
                        for ci in range(CT):
                            pt = pst_pool.tile([128, 128], f32)
                            nc.tensor.transpose(
                                pt[:], xt_in[:, ci * 128:(ci + 1) * 128],
                                ident[:])
                            nc.vector.tensor_copy(
                                xT[ci][:, ni * 128:(ni + 1) * 128], pt[:])

                    # ---- phase A1: qk^T = w_qk.T @ x^T ----
                    for fi in range(FT):
                        for ch in range(NCH):
                            pqk = psqk_pool.tile([128, 512], f32)
                            for ci in range(CT):
                                nc.tensor.matmul(
                                    pqk[:],
                                    r(wq[ci][:, fi * 128:(fi + 1) * 128]),
                                    r(xT[ci][:, ch * 512:(ch + 1) * 512]),
                                    start=(ci == 0), stop=(ci == CT - 1))
                            nc.vector.tensor_copy(
                                qkT[fi][:, ch * 512:(ch + 1) * 512], pqk[:])

                    # ---- phase A2: v natural = (x^T).T @ w_v, + ones col ----
                    for ni in range(NT):
                        nc.vector.tensor_copy(vnat[ni][:, :, D], vones_f32[:])
                        for vc in range(2):
                            pv = psv_pool.tile([128, VCH], f32)
                            for ci in range(CT):
                                nc.tensor.matmul(
                                    pv[:],
                                    r(xT[ci][:, ni * 128:(ni + 1) * 128]),
                                    r(wq[ci][:, FQK + vc * VCH:
                                             FQK + (vc + 1) * VCH]),
                                    start=(ci == 0), stop=(ci == CT - 1))
                            nc.vector.tensor_copy(
                                vnat[ni][:, vc * 6:(vc + 1) * 6, 0:D],
                                pv[:].rearrange("p (h d) -> p h d", d=D))

                # ---- phase B: attention per head ----
                with tc.tile_pool(name="wp", bufs=1) as wp_pool:
                    wp = [wp_pool.tile([128, C], f32r, tag=f"wp{ci}", name=f"wp{ci}")
                          for ci in range(CT)]
                    for ci in range(CT):
                        nc.scalar.dma_start(
                            wp[ci][:], wproj_d[ci * 128:(ci + 1) * 128, :])

                  # (indent fix below)
                    attn_pools = (
                        tc.tile_pool(name="rc", bufs=4),
                        tc.tile_pool(name="exp", bufs=10),
                        tc.tile_pool(name="pss", bufs=3, space="PSUM"),
                        tc.tile_pool(name="pso", bufs=3, space="PSUM"),
                        tc.tile_pool(name="psb", bufs=2, space="PSUM"),
                    )
                    rc_pool, exp_pool, pss_pool, pso_pool, psb_pool = [
                        p.__enter__() for p in attn_pools]

                    for h in range(H):
                        qrow = (h % 2) * D
                        qT_h = qkT[h // 2][qrow:qrow + D, :]
                        kT_h = qkT[6 + h // 2][qrow:qrow + D, :]

                        exp_t = []
                        for mi in range(NT):
                            et = exp_pool.tile([128, N], f16, tag="exp", name=f"exp{mi}")
                            exp_t.append(et)
                            for ch in range(NCH):
                                ps = pss_pool.tile([128, 512], f32)
                                nc.tensor.matmul(
                                    ps[:],
                                    r(kT_h[:, mi * 128:(mi + 1) * 128]),
                                    r(qT_h[:, ch * 512:(ch + 1) * 512]),
                                    start=True, stop=True)
                                nc.scalar.activation(
                                    et[:, ch * 512:(ch + 1) * 512], ps[:],
                                    mybir.ActivationFunctionType.Exp,
                                    scale=SCALE)

                        orow = (h % 2) * D
                        for ch in range(NCH):
                            po = pso_pool.tile([D + 1, 512], f32)
                            for mi in range(NT):
                                nc.tensor.matmul(
                                    po[:],
                                    r(vnat[mi][:, h, :]),
                                    r(exp_t[mi][:, ch * 512:(ch + 1) * 512]),
                                    start=(mi == 0), stop=(mi == NT - 1))
                            rs = rc_pool.tile([1, 512], f32, tag="rs",
                                              name=f"rs{h}_{ch}", bufs=2)
                            nc.vector.tensor_copy(rs[:], po[D:D + 1, :])
                            rcf = rc_pool.tile([1, 512], f32, tag="rcf",
                                               name=f"rcf{h}_{ch}", bufs=2)
                            nc.vector.reciprocal_approx_fast(rcf[:], rs[:])
                            rc = rc_pool.tile([1, 512], f32r, tag="rc",
                                              name=f"rc{h}_{ch}", bufs=2)
                            nc.vector.tensor_copy(rc[:], rcf[:])
                            pb = psb_pool.tile([128, 512], f32)
                            nc.tensor.matmul(
                                pb[:], r(ones_sb[:]), r(rc[:]),
                                start=True, stop=True)
                            ou = rc_pool.tile([D, 512], f32, tag="ou",
                                              name=f"ou{h}_{ch}", bufs=3)
                            nc.vector.tensor_copy(ou[:], po[0:D, :])
                            nc.vector.tensor_mul(
                                onorm[h // 2][orow:orow + D,
                                              ch * 512:(ch + 1) * 512],
                                ou[:], pb[0:D, :])

                    for p in reversed(attn_pools):
                        p.__exit__(None, None, None)

                    # ---- phase C: final = (out^T).T @ w_proj + b ----
                    with tc.tile_pool(name="fin", bufs=3) as fin_pool, \
                         tc.tile_pool(name="psf", bufs=2, space="PSUM") as psf_pool:
                        for ni in range(NT):
                            fin = fin_pool.tile([128, C], f32)
                            for fc in range(2):
                                pf = psf_pool.tile([128, VCH], f32)
                                for ci in range(CT):
                                    nc.tensor.matmul(
                                        pf[:],
                                        r(onorm[ci][:, ni * 128:(ni + 1) * 128]),
                                        r(wp[ci][:, fc * VCH:(fc + 1) * VCH]),
                                        start=(ci == 0), stop=(ci == CT - 1))
                                nc.vector.tensor_add(
                                    fin[:, fc * VCH:(fc + 1) * VCH], pf[:],
                                    bias_sb[:, fc * VCH:(fc + 1) * VCH])
                            nc.sync.dma_start(
                                out_d[ni * 128:(ni + 1) * 128, :], fin[:])

    nc.compile()
    return nc


def _get_compiled():
    global _compiled
    if _compiled is None:
        _compiled = _build()
    return _compiled


def _run(x, w_qkv, w_proj, b_proj, **kwargs):
    from concourse.bass_utils import run_bass_kernel_spmd

    x = np.asarray(x, dtype=np.float32)
    w_qkv = np.ascontiguousarray(np.asarray(w_qkv, dtype=np.float32))
    w_proj = np.ascontiguousarray(np.asarray(w_proj, dtype=np.float32))
    b_bcast = np.ascontiguousarray(
        np.broadcast_to(np.asarray(b_proj, dtype=np.float32), (128, C)))

    nc = _get_compiled()
    in_maps = [
        {"x": np.ascontiguousarray(x[b]), "w_qkv": w_qkv,
         "w_proj": w_proj, "b_bcast": b_bcast}
        for b in range(B)
    ]
    return run_bass_kernel_spmd(nc, in_maps, core_ids=list(range(B)), **kwargs)


def kernel(x, w_qkv, w_proj, b_proj, **_):
    res = _run(x, w_qkv, w_proj, b_proj)
    return np.stack([res.results[b]["out"] for b in range(B)], axis=0)


# revision 18
# speedup vs baseline: 1.4439x; 1.2104x over previous
"""Multi-head attention (B=8, N=1024, C=768, H=12, D=64) on 8 TRN2 NeuronCores.

Sharding: pure data parallel — one batch element per core, weights replicated,
no collectives. Each core computes its full attention block.

On-chip layout (per core), fp16 operands / fp32 PSUM accumulation:
  - host casts x / w_qkv / w_proj to fp16; x^T arrives via DMA xbar transpose
    (2-byte dtype) — no PE transposes at all.
  - qk^T [1536, N] = w_qk.T @ x^T (transposed activations; the q half is
    pre-scaled by 1/sqrt(D) during the PSUM->SBUF cast so exp needs no scale).
  - v [N, 768] natural = (x^T).T @ w_v, stored fp16 with a ones column per
    head ([128, 12, 65]) so attn@v also produces the softmax denominator.
  - heads processed in even/odd pairs: the pair's score matmuls use PE row
    groups 0-1 vs 2-3 (K=64 at base partitions 0/64) and can run
    concurrently; score output is a single-bank fp16 [128, 1024] PSUM tile
    (no accumulation), consumed by one wide ACT exp per m-tile.
  - attn@v: 4 accumulation chains (2 heads x 2 n-chunks) interleaved over
    m-tiles so consecutive matmuls target different PSUM banks.
  - softmax denominators: fp32 rowsum row -> base-0 copy ->
    reciprocal_approx_fast (~18 bits, plenty for well-conditioned sums) ->
    fp16 -> PE broadcast (ones[1,128].T @ recip[1,512]) -> DVE multiply.
  - final = (out^T).T @ w_proj + b: out^T stationary flips the result back to
    natural [N, C] so the output DMA is contiguous fp32.
"""

import numpy as np

B, N, C = 8, 1024, 768
H, D = 12, 64
F3 = 3 * C          # 2304
FQK = 2 * C         # 1536
SCALE = D ** -0.5   # 0.125
NT = N // 128       # 8 n-tiles / m-tiles
CT = C // 128       # 6 c-tiles
FT = FQK // 128     # 12 qk feature tiles
NCH = N // 512      # 2 psum chunks over n
VCH = 384           # v / proj free chunk (C = 2*384)

_compiled = None


def _build():
    import concourse.mybir as mybir
    import concourse.tile as tile
    from concourse import bacc

    f32 = mybir.dt.float32
    f16 = mybir.dt.float16

    nc = bacc.Bacc("TRN2", target_bir_lowering=False, debug=False)

    x_d = nc.dram_tensor("x", [N, C], f16, kind="ExternalInput").ap()
    wqkv_d = nc.dram_tensor("w_qkv", [C, F3], f16, kind="ExternalInput").ap()
    wproj_d = nc.dram_tensor("w_proj", [C, C], f16, kind="ExternalInput").ap()
    bias_d = nc.dram_tensor("b_bcast", [128, C], f32, kind="ExternalInput").ap()
    out_d = nc.dram_tensor("out", [N, C], f32, kind="ExternalOutput").ap()

    with tile.TileContext(nc) as tc:
        with tc.tile_pool(name="const", bufs=1) as const_pool:
            ones_f32 = const_pool.tile([1, 128], f32)
            nc.gpsimd.memset(ones_f32[:], 1.0)
            ones_sb = const_pool.tile([1, 128], f16)
            nc.vector.tensor_copy(ones_sb[:], ones_f32[:])
            vones_f32 = const_pool.tile([128, H], f32)
            nc.gpsimd.memset(vones_f32[:], 1.0)
            bias_sb = const_pool.tile([128, C], f32)
            nc.scalar.dma_start(bias_sb[:], bias_d)

            # ---- persistent activations ----
            with tc.tile_pool(name="acts", bufs=1) as acts:
                xT = [acts.tile([128, N], f16, tag=f"xT{ci}", name=f"xT{ci}")
                      for ci in range(CT)]
                qkT = [acts.tile([128, N], f16, tag=f"qkT{fi}", name=f"qkT{fi}")
                       for fi in range(FT)]
                vnat = [acts.tile([128, H, D + 1], f16, tag=f"v{ni}",
                                  name=f"v{ni}") for ni in range(NT)]
                onorm = [acts.tile([128, N], f16, tag=f"on{ci}", name=f"on{ci}")
                         for ci in range(CT)]

                # ---- phase 0: x^T via DMA xbar transpose ----
                for ci in range(CT):
                    nc.sync.dma_start(
                        xT[ci][:], x_d[:, ci * 128:(ci + 1) * 128],
                        transpose=True)

                with tc.tile_pool(name="wq", bufs=1) as wq_pool, \
                     tc.tile_pool(name="psqk", bufs=2, space="PSUM") as psqk_pool, \
                     tc.tile_pool(name="psv", bufs=2, space="PSUM") as psv_pool:
                    wq = [wq_pool.tile([128, F3], f16, tag=f"wq{ci}",
                                       name=f"wq{ci}") for ci in range(CT)]
                    for ci in range(CT):
                        nc.scalar.dma_start(
                            wq[ci][:], wqkv_d[ci * 128:(ci + 1) * 128, :])

                    # ---- phase A1: qk^T = w_qk.T @ x^T ----
                    for fi in range(FT):
                        for ch in range(NCH):
                            pqk = psqk_pool.tile([128, 512], f32)
                            for ci in range(CT):
                                nc.tensor.matmul(
                                    pqk[:],
                                    wq[ci][:, fi * 128:(fi + 1) * 128],
                                    xT[ci][:, ch * 512:(ch + 1) * 512],
                                    start=(ci == 0), stop=(ci == CT - 1))
                            if fi < 6:
                                # q half: fold in the 1/sqrt(D) scale
                                nc.vector.tensor_scalar_mul(
                                    qkT[fi][:, ch * 512:(ch + 1) * 512],
                                    pqk[:], SCALE)
                            else:
                                nc.vector.tensor_copy(
                                    qkT[fi][:, ch * 512:(ch + 1) * 512],
                                    pqk[:])

                    # ---- phase A2: v natural = (x^T).T @ w_v, + ones col ----
                    for ni in range(NT):
                        nc.vector.tensor_copy(vnat[ni][:, :, D], vones_f32[:])
                        for vc in range(2):
                            pv = psv_pool.tile([128, VCH], f32)
                            for ci in range(CT):
                                nc.tensor.matmul(
                                    pv[:],
                                    xT[ci][:, ni * 128:(ni + 1) * 128],
                                    wq[ci][:, FQK + vc * VCH:
                                           FQK + (vc + 1) * VCH],
                                    start=(ci == 0), stop=(ci == CT - 1))
                            nc.vector.tensor_copy(
                                vnat[ni][:, vc * 6:(vc + 1) * 6, 0:D],
                                pv[:].rearrange("p (h d) -> p h d", d=D))

                # ---- phase B: attention, head pairs ----
                with tc.tile_pool(name="wp", bufs=1) as wp_pool:
                    wp = [wp_pool.tile([128, C], f16, tag=f"wp{ci}",
                                       name=f"wp{ci}") for ci in range(CT)]
                    for ci in range(CT):
                        nc.scalar.dma_start(
                            wp[ci][:], wproj_d[ci * 128:(ci + 1) * 128, :])

                    attn_pools = (
                        tc.tile_pool(name="rc", bufs=2),
                        tc.tile_pool(name="exp", bufs=8),
                        tc.tile_pool(name="pss", bufs=2, space="PSUM"),
                        tc.tile_pool(name="pso", bufs=8, space="PSUM"),
                    )
                    rc_pool, exp_pool, pss_pool, pso_pool = [
                        p.__enter__() for p in attn_pools]

                    for j in range(H // 2):
                        pair = (2 * j, 2 * j + 1)
                        # 4 accumulation chains: (head, chunk)
                        po = {}
                        for h in pair:
                            for ch in range(NCH):
                                po[(h, ch)] = pso_pool.tile(
                                    [D + 1, 512], f32, tag="po", bufs=4,
                                    name=f"po{h}_{ch}")
                        for mi in range(NT):
                            ets = {}
                            for h in pair:
                                qrow = (h % 2) * D
                                ps = pss_pool.tile([128, N], f32, tag="pss",
                                                   name=f"pss{h}_{mi}")
                                for ch in range(NCH):
                                    nc.tensor.matmul(
                                        ps[:, ch * 512:(ch + 1) * 512],
                                        qkT[6 + h // 2][qrow:qrow + D,
                                                        mi * 128:(mi + 1) * 128],
                                        qkT[h // 2][qrow:qrow + D,
                                                    ch * 512:(ch + 1) * 512],
                                        start=True, stop=True)
                                et = exp_pool.tile([128, N], f16, tag="exp",
                                                   name=f"exp{h}_{mi}")
                                nc.scalar.activation(
                                    et[:], ps[:],
                                    mybir.ActivationFunctionType.Exp)
                                ets[h] = et
                            for h in pair:
                                for ch in range(NCH):
                                    nc.tensor.matmul(
                                        po[(h, ch)][:],
                                        vnat[mi][:, h, :],
                                        ets[h][:, ch * 512:(ch + 1) * 512],
                                        start=(mi == 0), stop=(mi == NT - 1))
                        for h in pair:
                            orow = (h % 2) * D
                            for ch in range(NCH):
                                p = po[(h, ch)]
                                rs = rc_pool.tile([1, 512], f32, tag="rs",
                                                  name=f"rs{h}_{ch}", bufs=3)
                                nc.vector.tensor_copy(rs[:], p[D:D + 1, :])
                                rcf = rc_pool.tile([1, 512], f32, tag="rcf",
                                                   name=f"rcf{h}_{ch}", bufs=3)
                                nc.vector.reciprocal_approx_fast(rcf[:], rs[:])
                                rc = rc_pool.tile([1, 512], f16, tag="rc",
                                                  name=f"rc{h}_{ch}", bufs=3)
                                nc.vector.tensor_copy(rc[:], rcf[:])
                                ou = rc_pool.tile([D, 512], f32, tag="ou",
                                                  name=f"ou{h}_{ch}", bufs=3)
                                nc.vector.tensor_copy(ou[:], p[0:D, :])
                                pb = pso_pool.tile([128, 512], f32, tag="po",
                                                   bufs=4, name=f"pb{h}_{ch}")
                                nc.tensor.matmul(
                                    pb[:], ones_sb[:], rc[:],
                                    start=True, stop=True)
                                nc.vector.tensor_mul(
                                    onorm[h // 2][orow:orow + D,
                                                  ch * 512:(ch + 1) * 512],
                                    ou[:], pb[0:D, :])

                    for p in reversed(attn_pools):
                        p.__exit__(None, None, None)

                    # ---- phase C: final = (out^T).T @ w_proj + b ----
                    with tc.tile_pool(name="fin", bufs=3) as fin_pool, \
                         tc.tile_pool(name="psf", bufs=2, space="PSUM") as psf_pool:
                        for ni in range(NT):
                            fin = fin_pool.tile([128, C], f32)
                            for fc in range(2):
                                pf = psf_pool.tile([128, VCH], f32)
                                for ci in range(CT):
                                    nc.tensor.matmul(
                                        pf[:],
                                        onorm[ci][:, ni * 128:(ni + 1) * 128],
                                        wp[ci][:, fc * VCH:(fc + 1) * VCH],
                                        start=(ci == 0), stop=(ci == CT - 1))
                                nc.vector.tensor_add(
                                    fin[:, fc * VCH:(fc + 1) * VCH], pf[:],
                                    bias_sb[:, fc * VCH:(fc + 1) * VCH])
                            nc.sync.dma_start(
                                out_d[ni * 128:(ni + 1) * 128, :], fin[:])

    nc.compile()
    return nc


def _get_compiled():
    global _compiled
    if _compiled is None:
        _compiled = _build()
    return _compiled


def _run(x, w_qkv, w_proj, b_proj, **kwargs):
    from concourse.bass_utils import run_bass_kernel_spmd

    x = np.asarray(x, dtype=np.float32).astype(np.float16)
    w_qkv = np.ascontiguousarray(
        np.asarray(w_qkv, dtype=np.float32).astype(np.float16))
    w_proj = np.ascontiguousarray(
        np.asarray(w_proj, dtype=np.float32).astype(np.float16))
    b_bcast = np.ascontiguousarray(
        np.broadcast_to(np.asarray(b_proj, dtype=np.float32), (128, C)))

    nc = _get_compiled()
    in_maps = [
        {"x": np.ascontiguousarray(x[b]), "w_qkv": w_qkv,
         "w_proj": w_proj, "b_bcast": b_bcast}
        for b in range(B)
    ]
    return run_bass_kernel_spmd(nc, in_maps, core_ids=list(range(B)), **kwargs)


def kernel(x, w_qkv, w_proj, b_proj, **_):
    res = _run(x, w_qkv, w_proj, b_proj)
    return np.stack([res.results[b]["out"] for b in range(B)], axis=0)


# revision 19
# speedup vs baseline: 1.5642x; 1.0833x over previous
"""Multi-head attention (B=8, N=1024, C=768, H=12, D=64) on 8 TRN2 NeuronCores.

Sharding: pure data parallel — one batch element per core, weights replicated,
no collectives. Each core computes its full attention block.

On-chip layout (per core), fp16 operands / fp32 PSUM accumulation:
  - host casts x / w_qkv / w_proj to fp16; x^T arrives via DMA xbar transpose
    (2-byte dtype) — no PE transposes at all.
  - qk^T [1536, N] = w_qk.T @ x^T (transposed activations; the q half is
    pre-scaled by 1/sqrt(D) during the PSUM->SBUF cast so exp needs no scale).
  - v [N, 768] natural = (x^T).T @ w_v, stored fp16 with a ones column per
    head ([128, 12, 65]) so attn@v also produces the softmax denominator.
  - heads processed in even/odd pairs: the pair's score matmuls use PE row
    groups 0-1 vs 2-3 (K=64 at base partitions 0/64) and can run
    concurrently; score output is a single-bank fp16 [128, 1024] PSUM tile
    (no accumulation), consumed by one wide ACT exp per m-tile.
  - attn@v: 4 accumulation chains (2 heads x 2 n-chunks) interleaved over
    m-tiles so consecutive matmuls target different PSUM banks.
  - softmax denominators: fp32 rowsum row -> base-0 copy ->
    reciprocal_approx_fast (~18 bits, plenty for well-conditioned sums) ->
    fp16 -> PE broadcast (ones[1,128].T @ recip[1,512]) -> DVE multiply.
  - final = (out^T).T @ w_proj + b: out^T stationary flips the result back to
    natural [N, C] so the output DMA is contiguous fp32.
"""

import numpy as np

B, N, C = 8, 1024, 768
H, D = 12, 64
F3 = 3 * C          # 2304
FQK = 2 * C         # 1536
SCALE = D ** -0.5   # 0.125
NT = N // 128       # 8 n-tiles / m-tiles
CT = C // 128       # 6 c-tiles
FT = FQK // 128     # 12 qk feature tiles
NCH = N // 512      # 2 psum chunks over n
VCH = 384           # v / proj free chunk (C = 2*384)

_compiled = None


def _build():
    import concourse.mybir as mybir
    import concourse.tile as tile
    from concourse import bacc

    f32 = mybir.dt.float32
    f16 = mybir.dt.float16

    nc = bacc.Bacc("TRN2", target_bir_lowering=False, debug=False)

    x_d = nc.dram_tensor("x", [N, C], f16, kind="ExternalInput").ap()
    wqkv_d = nc.dram_tensor("w_qkv", [C, F3], f16, kind="ExternalInput").ap()
    wproj_d = nc.dram_tensor("w_proj", [C, C], f16, kind="ExternalInput").ap()
    bias_d = nc.dram_tensor("b_bcast", [128, C], f32, kind="ExternalInput").ap()
    out_d = nc.dram_tensor("out", [N, C], f32, kind="ExternalOutput").ap()

    with tile.TileContext(nc) as tc:
        with tc.tile_pool(name="const", bufs=1) as const_pool:
            ones_f32 = const_pool.tile([1, 128], f32)
            nc.gpsimd.memset(ones_f32[:], 1.0)
            ones_sb = const_pool.tile([1, 128], f16)
            nc.vector.tensor_copy(ones_sb[:], ones_f32[:])
            vones_f32 = const_pool.tile([128, H], f32)
            nc.gpsimd.memset(vones_f32[:], 1.0)
            bias_sb = const_pool.tile([128, C], f32)
            nc.scalar.dma_start(bias_sb[:], bias_d)

            # ---- persistent activations ----
            with tc.tile_pool(name="acts", bufs=1) as acts:
                xT = [acts.tile([128, N], f16, tag=f"xT{ci}", name=f"xT{ci}")
                      for ci in range(CT)]
                qkT = [acts.tile([128, N], f16, tag=f"qkT{fi}", name=f"qkT{fi}")
                       for fi in range(FT)]
                vnat = [acts.tile([128, H, D + 1], f16, tag=f"v{ni}",
                                  name=f"v{ni}") for ni in range(NT)]
                onorm = [acts.tile([128, N], f16, tag=f"on{ci}", name=f"on{ci}")
                         for ci in range(CT)]

                # ---- phase 0: x^T via DMA xbar transpose ----
                for ci in range(CT):
                    nc.sync.dma_start(
                        xT[ci][:], x_d[:, ci * 128:(ci + 1) * 128],
                        transpose=True)

                with tc.tile_pool(name="wq", bufs=1) as wq_pool, \
                     tc.tile_pool(name="wp", bufs=1) as wp_pool, \
                     tc.tile_pool(name="acc", bufs=2, space="PSUM") as acc_pool:
                    wq = [wq_pool.tile([128, F3], f16, tag=f"wq{ci}",
                                       name=f"wq{ci}") for ci in range(CT)]
                    for ci in range(CT):
                        eng = nc.scalar if ci < 5 else nc.sync
                        eng.dma_start(
                            wq[ci][:], wqkv_d[ci * 128:(ci + 1) * 128, :])
                    wp = [wp_pool.tile([128, C], f16, tag=f"wp{ci}",
                                       name=f"wp{ci}") for ci in range(CT)]
                    for ci in range(CT):
                        nc.scalar.dma_start(
                            wp[ci][:], wproj_d[ci * 128:(ci + 1) * 128, :])

                    def qk_proj(fi):
                        for ch in range(NCH):
                            pqk = acc_pool.tile([128, 512], f32, tag="acc",
                                                name=f"pqk{fi}_{ch}")
                            for ci in range(CT):
                                nc.tensor.matmul(
                                    pqk[:],
                                    wq[ci][:, fi * 128:(fi + 1) * 128],
                                    xT[ci][:, ch * 512:(ch + 1) * 512],
                                    start=(ci == 0), stop=(ci == CT - 1))
                            if fi < 6:
                                # q half: fold in the 1/sqrt(D) scale
                                nc.vector.tensor_scalar_mul(
                                    qkT[fi][:, ch * 512:(ch + 1) * 512],
                                    pqk[:], SCALE)
                            else:
                                nc.vector.tensor_copy(
                                    qkT[fi][:, ch * 512:(ch + 1) * 512],
                                    pqk[:])

                    # first pair's qk tiles
                    qk_proj(0)
                    qk_proj(6)

                    # ---- v natural = (x^T).T @ w_v, + ones col ----
                    for ni in range(NT):
                        nc.vector.tensor_copy(vnat[ni][:, :, D], vones_f32[:])
                        for vc in range(2):
                            pv = acc_pool.tile([128, VCH], f32, tag="acc",
                                               name=f"pv{ni}_{vc}")
                            for ci in range(CT):
                                nc.tensor.matmul(
                                    pv[:],
                                    xT[ci][:, ni * 128:(ni + 1) * 128],
                                    wq[ci][:, FQK + vc * VCH:
                                           FQK + (vc + 1) * VCH],
                                    start=(ci == 0), stop=(ci == CT - 1))
                            nc.vector.tensor_copy(
                                vnat[ni][:, vc * 6:(vc + 1) * 6, 0:D],
                                pv[:].rearrange("p (h d) -> p h d", d=D))

                    # ---- attention, head pairs, qk for pair j+1 interleaved
                    attn_pools = (
                        tc.tile_pool(name="rc", bufs=2),
                        tc.tile_pool(name="exp", bufs=18),
                        tc.tile_pool(name="pss", bufs=2, space="PSUM"),
                        tc.tile_pool(name="pso", bufs=2, space="PSUM"),
                    )
                    rc_pool, exp_pool, pss_pool, pso_pool = [
                        p.__enter__() for p in attn_pools]

                    for j in range(H // 2):
                        if j + 1 < H // 2:
                            qk_proj(j + 1)
                            qk_proj(6 + j + 1)
                        pair = (2 * j, 2 * j + 1)
                        exp_t = {h: [] for h in pair}
                        for mi in range(NT):
                            for h in pair:
                                qrow = (h % 2) * D
                                ps = pss_pool.tile([128, N], f32, tag="pss",
                                                   name=f"pss{h}_{mi}")
                                for ch in range(NCH):
                                    nc.tensor.matmul(
                                        ps[:, ch * 512:(ch + 1) * 512],
                                        qkT[6 + h // 2][qrow:qrow + D,
                                                        mi * 128:(mi + 1) * 128],
                                        qkT[h // 2][qrow:qrow + D,
                                                    ch * 512:(ch + 1) * 512],
                                        start=True, stop=True)
                                et = exp_pool.tile([128, N], f16, tag="exp",
                                                   name=f"exp{h}_{mi}")
                                nc.scalar.activation(
                                    et[:], ps[:],
                                    mybir.ActivationFunctionType.Exp)
                                exp_t[h].append(et)
                        for ch in range(NCH):
                            po = {}
                            for h in pair:
                                po[h] = pso_pool.tile(
                                    [D + 1, 512], f32, tag="po",
                                    name=f"po{h}_{ch}")
                            for mi in range(NT):
                                for h in pair:
                                    nc.tensor.matmul(
                                        po[h][:],
                                        vnat[mi][:, h, :],
                                        exp_t[h][mi][:,
                                                     ch * 512:(ch + 1) * 512],
                                        start=(mi == 0), stop=(mi == NT - 1))
                            for h in pair:
                                orow = (h % 2) * D
                                p = po[h]
                                rs = rc_pool.tile([1, 512], f32, tag="rs",
                                                  name=f"rs{h}_{ch}", bufs=3)
                                nc.vector.tensor_copy(rs[:], p[D:D + 1, :])
                                rcf = rc_pool.tile([1, 512], f32, tag="rcf",
                                                   name=f"rcf{h}_{ch}", bufs=3)
                                nc.vector.reciprocal_approx_fast(rcf[:], rs[:])
                                rc = rc_pool.tile([1, 512], f16, tag="rc",
                                                  name=f"rc{h}_{ch}", bufs=3)
                                nc.vector.tensor_copy(rc[:], rcf[:])
                                ou = rc_pool.tile([D, 512], f32, tag="ou",
                                                  name=f"ou{h}_{ch}", bufs=3)
                                nc.vector.tensor_copy(ou[:], p[0:D, :])
                                pb = pso_pool.tile([128, 512], f32, tag="po",
                                                   name=f"pb{h}_{ch}")
                                nc.tensor.matmul(
                                    pb[:], ones_sb[:], rc[:],
                                    start=True, stop=True)
                                nc.vector.tensor_mul(
                                    onorm[h // 2][orow:orow + D,
                                                  ch * 512:(ch + 1) * 512],
                                    ou[:], pb[0:D, :])

                    for p in reversed(attn_pools):
                        p.__exit__(None, None, None)

                    # ---- phase C: final = (out^T).T @ w_proj + b ----
                    with tc.tile_pool(name="fin", bufs=3) as fin_pool:
                        for ni in range(NT):
                            fin = fin_pool.tile([128, C], f32)
                            for fc in range(2):
                                pf = acc_pool.tile([128, VCH], f32, tag="acc",
                                                   name=f"pf{ni}_{fc}")
                                for ci in range(CT):
                                    nc.tensor.matmul(
                                        pf[:],
                                        onorm[ci][:, ni * 128:(ni + 1) * 128],
                                        wp[ci][:, fc * VCH:(fc + 1) * VCH],
                                        start=(ci == 0), stop=(ci == CT - 1))
                                nc.vector.tensor_add(
                                    fin[:, fc * VCH:(fc + 1) * VCH], pf[:],
                                    bias_sb[:, fc * VCH:(fc + 1) * VCH])
                            nc.sync.dma_start(
                                out_d[ni * 128:(ni + 1) * 128, :], fin[:])

    nc.compile()
    return nc


def _get_compiled():
    global _compiled
    if _compiled is None:
        _compiled = _build()
    return _compiled


def _run(x, w_qkv, w_proj, b_proj, **kwargs):
    from concourse.bass_utils import run_bass_kernel_spmd

    x = np.asarray(x, dtype=np.float32).astype(np.float16)
    w_qkv = np.ascontiguousarray(
        np.asarray(w_qkv, dtype=np.float32).astype(np.float16))
    w_proj = np.ascontiguousarray(
        np.asarray(w_proj, dtype=np.float32).astype(np.float16))
    b_bcast = np.ascontiguousarray(
        np.broadcast_to(np.asarray(b_proj, dtype=np.float32), (128, C)))

    nc = _get_compiled()
    in_maps = [
        {"x": np.ascontiguousarray(x[b]), "w_qkv": w_qkv,
         "w_proj": w_proj, "b_bcast": b_bcast}
        for b in range(B)
    ]
    return run_bass_kernel_spmd(nc, in_maps, core_ids=list(range(B)), **kwargs)


def kernel(x, w_qkv, w_proj, b_proj, **_):
    res = _run(x, w_qkv, w_proj, b_proj)
    return np.stack([res.results[b]["out"] for b in range(B)], axis=0)


# revision 20
# speedup vs baseline: 1.5738x; 1.0062x over previous
"""Multi-head attention (B=8, N=1024, C=768, H=12, D=64) on 8 TRN2 NeuronCores.

Sharding: pure data parallel — one batch element per core, weights replicated,
no collectives. Each core computes its full attention block.

On-chip layout (per core), fp16 operands / fp32 PSUM accumulation:
  - host casts x / w_qkv / w_proj to fp16; x^T arrives via DMA xbar transpose
    (2-byte dtype) — no PE transposes at all.
  - qk^T [1536, N] = w_qk.T @ x^T (transposed activations; the q half is
    pre-scaled by 1/sqrt(D) during the PSUM->SBUF cast so exp needs no scale).
  - v [N, 768] natural = (x^T).T @ w_v, stored fp16 with a ones column per
    head ([128, 12, 65]) so attn@v also produces the softmax denominator.
  - heads processed in even/odd pairs: the pair's score matmuls use PE row
    groups 0-1 vs 2-3 (K=64 at base partitions 0/64) and can run
    concurrently; score output is a single-bank fp16 [128, 1024] PSUM tile
    (no accumulation), consumed by one wide ACT exp per m-tile.
  - attn@v: 4 accumulation chains (2 heads x 2 n-chunks) interleaved over
    m-tiles so consecutive matmuls target different PSUM banks.
  - softmax denominators: fp32 rowsum row -> base-0 copy ->
    reciprocal_approx_fast (~18 bits, plenty for well-conditioned sums) ->
    fp16 -> PE broadcast (ones[1,128].T @ recip[1,512]) -> DVE multiply.
  - final = (out^T).T @ w_proj + b: out^T stationary flips the result back to
    natural [N, C] so the output DMA is contiguous fp32.
"""

import numpy as np

B, N, C = 8, 1024, 768
H, D = 12, 64
F3 = 3 * C          # 2304
FQK = 2 * C         # 1536
SCALE = D ** -0.5   # 0.125
NT = N // 128       # 8 n-tiles / m-tiles
CT = C // 128       # 6 c-tiles
FT = FQK // 128     # 12 qk feature tiles
NCH = N // 512      # 2 psum chunks over n
VCH = 384           # v / proj free chunk (C = 2*384)

_compiled = None


def _build():
    import concourse.mybir as mybir
    import concourse.tile as tile
    from concourse import bacc

    f32 = mybir.dt.float32
    f16 = mybir.dt.float16

    nc = bacc.Bacc("TRN2", target_bir_lowering=False, debug=False)

    x_d = nc.dram_tensor("x", [N, C], f16, kind="ExternalInput").ap()
    wqkv_d = nc.dram_tensor("w_qkv", [C, F3], f16, kind="ExternalInput").ap()
    wproj_d = nc.dram_tensor("w_proj", [C, C], f16, kind="ExternalInput").ap()
    bias_d = nc.dram_tensor("b_bcast", [128, C], f32, kind="ExternalInput").ap()
    out_d = nc.dram_tensor("out", [N, C], f32, kind="ExternalOutput").ap()

    with tile.TileContext(nc) as tc:
        with tc.tile_pool(name="const", bufs=1) as const_pool:
            ones_f32 = const_pool.tile([1, 128], f32)
            nc.gpsimd.memset(ones_f32[:], 1.0)
            ones_sb = const_pool.tile([1, 128], f16)
            nc.vector.tensor_copy(ones_sb[:], ones_f32[:])
            vones_f32 = const_pool.tile([128, H], f32)
            nc.gpsimd.memset(vones_f32[:], 1.0)
            bias_sb = const_pool.tile([128, C], f32)
            nc.scalar.dma_start(bias_sb[:], bias_d)

            # ---- persistent activations ----
            with tc.tile_pool(name="acts", bufs=1) as acts:
                xT = [acts.tile([128, N], f16, tag=f"xT{ci}", name=f"xT{ci}")
                      for ci in range(CT)]
                qkT = [acts.tile([128, N], f16, tag=f"qkT{fi}", name=f"qkT{fi}")
                       for fi in range(FT)]
                vnat = [acts.tile([128, H, D + 1], f16, tag=f"v{ni}",
                                  name=f"v{ni}") for ni in range(NT)]
                onorm = [acts.tile([128, N], f16, tag=f"on{ci}", name=f"on{ci}")
                         for ci in range(CT)]

                # ---- phase 0: x^T via DMA xbar transpose ----
                for ci in range(CT):
                    nc.sync.dma_start(
                        xT[ci][:], x_d[:, ci * 128:(ci + 1) * 128],
                        transpose=True)

                with tc.tile_pool(name="wq", bufs=1) as wq_pool, \
                     tc.tile_pool(name="wp", bufs=1) as wp_pool, \
                     tc.tile_pool(name="acc", bufs=2, space="PSUM") as acc_pool:
                    wq = [wq_pool.tile([128, F3], f16, tag=f"wq{ci}",
                                       name=f"wq{ci}") for ci in range(CT)]
                    for ci in range(CT):
                        eng = nc.scalar if ci < 5 else nc.sync
                        eng.dma_start(
                            wq[ci][:], wqkv_d[ci * 128:(ci + 1) * 128, :])
                    wp = [wp_pool.tile([128, C], f16, tag=f"wp{ci}",
                                       name=f"wp{ci}") for ci in range(CT)]
                    for ci in range(CT):
                        nc.scalar.dma_start(
                            wp[ci][:], wproj_d[ci * 128:(ci + 1) * 128, :])

                    def qk_proj(fi):
                        pqk = [acc_pool.tile([128, 512], f32, tag="acc",
                                             name=f"pqk{fi}_{ch}")
                               for ch in range(NCH)]
                        for ci in range(CT):
                            for ch in range(NCH):
                                nc.tensor.matmul(
                                    pqk[ch][:],
                                    wq[ci][:, fi * 128:(fi + 1) * 128],
                                    xT[ci][:, ch * 512:(ch + 1) * 512],
                                    start=(ci == 0), stop=(ci == CT - 1))
                        for ch in range(NCH):
                            if fi < 6:
                                # q half: fold in the 1/sqrt(D) scale
                                nc.vector.tensor_scalar_mul(
                                    qkT[fi][:, ch * 512:(ch + 1) * 512],
                                    pqk[ch][:], SCALE)
                            else:
                                nc.vector.tensor_copy(
                                    qkT[fi][:, ch * 512:(ch + 1) * 512],
                                    pqk[ch][:])

                    def v_proj(ni):
                        nc.vector.tensor_copy(vnat[ni][:, :, D], vones_f32[:])
                        pv = [acc_pool.tile([128, VCH], f32, tag="acc",
                                            name=f"pv{ni}_{vc}")
                              for vc in range(2)]
                        for ci in range(CT):
                            for vc in range(2):
                                nc.tensor.matmul(
                                    pv[vc][:],
                                    xT[ci][:, ni * 128:(ni + 1) * 128],
                                    wq[ci][:, FQK + vc * VCH:
                                           FQK + (vc + 1) * VCH],
                                    start=(ci == 0), stop=(ci == CT - 1))
                        for vc in range(2):
                            nc.vector.tensor_copy(
                                vnat[ni][:, vc * 6:(vc + 1) * 6, 0:D],
                                pv[vc][:].rearrange("p (h d) -> p h d", d=D))

                    # first pair's qk tiles
                    qk_proj(0)
                    qk_proj(6)

                    # ---- attention, head pairs, qk for pair j+1 interleaved
                    attn_pools = (
                        tc.tile_pool(name="rc", bufs=2),
                        tc.tile_pool(name="exp", bufs=34),
                        tc.tile_pool(name="pss", bufs=2, space="PSUM"),
                        tc.tile_pool(name="pso", bufs=2, space="PSUM"),
                    )
                    rc_pool, exp_pool, pss_pool, pso_pool = [
                        p.__enter__() for p in attn_pools]

                    def scores_exp(j, exp_t):
                        pair = (2 * j, 2 * j + 1)
                        for mi in range(NT):
                            for h in pair:
                                qrow = (h % 2) * D
                                ps = pss_pool.tile([128, N], f32, tag="pss",
                                                   name=f"pss{h}_{mi}")
                                for ch in range(NCH):
                                    nc.tensor.matmul(
                                        ps[:, ch * 512:(ch + 1) * 512],
                                        qkT[6 + h // 2][qrow:qrow + D,
                                                        mi * 128:(mi + 1) * 128],
                                        qkT[h // 2][qrow:qrow + D,
                                                    ch * 512:(ch + 1) * 512],
                                        start=True, stop=True)
                                et = exp_pool.tile([128, N], f16, tag="exp",
                                                   name=f"exp{h}_{mi}")
                                nc.scalar.activation(
                                    et[:], ps[:],
                                    mybir.ActivationFunctionType.Exp)
                                exp_t[h].append(et)

                    def attnv_norm(j, exp_t):
                        pair = (2 * j, 2 * j + 1)
                        for ch in range(NCH):
                            po = {}
                            for h in pair:
                                po[h] = pso_pool.tile(
                                    [D + 1, 512], f32, tag="po",
                                    name=f"po{h}_{ch}")
                            for mi in range(NT):
                                for h in pair:
                                    nc.tensor.matmul(
                                        po[h][:],
                                        vnat[mi][:, h, :],
                                        exp_t[h][mi][:,
                                                     ch * 512:(ch + 1) * 512],
                                        start=(mi == 0), stop=(mi == NT - 1))
                            for h in pair:
                                orow = (h % 2) * D
                                p = po[h]
                                rs = rc_pool.tile([1, 512], f32, tag="rs",
                                                  name=f"rs{h}_{ch}", bufs=3)
                                nc.vector.tensor_copy(rs[:], p[D:D + 1, :])
                                rcf = rc_pool.tile([1, 512], f32, tag="rcf",
                                                   name=f"rcf{h}_{ch}", bufs=3)
                                nc.vector.reciprocal_approx_fast(rcf[:], rs[:])
                                rc = rc_pool.tile([1, 512], f16, tag="rc",
                                                  name=f"rc{h}_{ch}", bufs=3)
                                nc.vector.tensor_copy(rc[:], rcf[:])
                                ou = rc_pool.tile([D, 512], f32, tag="ou",
                                                  name=f"ou{h}_{ch}", bufs=3)
                                nc.vector.tensor_copy(ou[:], p[0:D, :])
                                pb = pso_pool.tile([128, 512], f32, tag="po",
                                                   name=f"pb{h}_{ch}")
                                nc.tensor.matmul(
                                    pb[:], ones_sb[:], rc[:],
                                    start=True, stop=True)
                                nc.vector.tensor_mul(
                                    onorm[h // 2][orow:orow + D,
                                                  ch * 512:(ch + 1) * 512],
                                    ou[:], pb[0:D, :])

                    exp_ts = {}
                    for j in range(H // 2):
                        exp_ts[j] = {2 * j: [], 2 * j + 1: []}
                        scores_exp(j, exp_ts[j])
                        if j == 0:
                            for ni in range(NT):
                                v_proj(ni)
                        if j + 1 < H // 2:
                            qk_proj(j + 1)
                            qk_proj(6 + j + 1)
                        if j >= 1:
                            attnv_norm(j - 1, exp_ts.pop(j - 1))
                    attnv_norm(H // 2 - 1, exp_ts.pop(H // 2 - 1))

                    for p in reversed(attn_pools):
                        p.__exit__(None, None, None)

                    # ---- phase C: final = (out^T).T @ w_proj + b ----
                    with tc.tile_pool(name="fin", bufs=3) as fin_pool:
                        for ni in range(NT):
                            fin = fin_pool.tile([128, C], f32)
                            pf = [acc_pool.tile([128, VCH], f32, tag="acc",
                                                name=f"pf{ni}_{fc}")
                                  for fc in range(2)]
                            for ci in range(CT):
                                for fc in range(2):
                                    nc.tensor.matmul(
                                        pf[fc][:],
                                        onorm[ci][:, ni * 128:(ni + 1) * 128],
                                        wp[ci][:, fc * VCH:(fc + 1) * VCH],
                                        start=(ci == 0), stop=(ci == CT - 1))
                            for fc in range(2):
                                nc.vector.tensor_add(
                                    fin[:, fc * VCH:(fc + 1) * VCH], pf[fc][:],
                                    bias_sb[:, fc * VCH:(fc + 1) * VCH])
                            nc.sync.dma_start(
                                out_d[ni * 128:(ni + 1) * 128, :], fin[:])

    nc.compile()
    return nc


def _get_compiled():
    global _compiled
    if _compiled is None:
        _compiled = _build()
    return _compiled


def _run(x, w_qkv, w_proj, b_proj, **kwargs):
    from concourse.bass_utils import run_bass_kernel_spmd

    x = np.asarray(x, dtype=np.float32).astype(np.float16)
    w_qkv = np.ascontiguousarray(
        np.asarray(w_qkv, dtype=np.float32).astype(np.float16))
    w_proj = np.ascontiguousarray(
        np.asarray(w_proj, dtype=np.float32).astype(np.float16))
    b_bcast = np.ascontiguousarray(
        np.broadcast_to(np.asarray(b_proj, dtype=np.float32), (128, C)))

    nc = _get_compiled()
    in_maps = [
        {"x": np.ascontiguousarray(x[b]), "w_qkv": w_qkv,
         "w_proj": w_proj, "b_bcast": b_bcast}
        for b in range(B)
    ]
    return run_bass_kernel_spmd(nc, in_maps, core_ids=list(range(B)), **kwargs)


def kernel(x, w_qkv, w_proj, b_proj, **_):
    res = _run(x, w_qkv, w_proj, b_proj)
    return np.stack([res.results[b]["out"] for b in range(B)], axis=0)
